# revision 15
# baseline (speedup 1.0000x reference)
"""PillarMaxPooling Trainium2 kernel (8 NeuronCores, SPMD).

Strategy
--------
Output (pillar) sharding: core c owns pillars [c*PPC, (c+1)*PPC).
Host-side prep is pure indexing/sharding work: points are routed to the
core that owns their pillar and packed into fixed per-pillar slot rows.
Pillars are stratified by point count into depth classes {1, 2, 4}
(8/16/32 slots); pillars with >32 points spill the excess into
"virtual pillar" entities combined on-device at the end.

BatchNorm folding: z = x @ (W * inv_std) + shift is one matmul via an
appended constant-1 feature carrying `shift`.  ReLU commutes with max,
and all-zero padding slots are exact neutral elements because
max(relu(a), 0) == relu(max(a, anything <= relu)).

Device program per core (identical program on all 8 cores):
  - xs  [128, NT*128] fp16 : slot features; entity (g,j) of depth d owns
        column-tiles [base(g) .. base(g)+d) at column j; each tile column
        packs 8 slots x 16 features down the 128-partition contraction.
  - w8  [128, 512] fp16 : block-diagonal folded weights; one matmul of an
        xs tile against w8 yields z for 8 slots x 64 channels (f32 PSUM).
  - per group: ACT relu-drains even PSUM tiles, DVE max-combines odd
        tiles, GPSIMD + DVE run the max tree -> [128, 64] f32 output rows.
"""

import os
import numpy as np

C_IN = 10
C_OUT = 64
N_CORES = 8
BN_EPS = 1e-3
F_PAD = 16            # features padded: 10 real + 1 const + 5 zero
MAX_SLOTS = 32        # slots per entity cap (depth class 4)
CHUNK_TILES = 32      # xs column-tiles per DMA chunk
D2_ACT_BOTH_MOD = 3   # every Nth depth-2 pair: ACT drains all four tiles

LAST_RESULTS = None
_PROGRAM_CACHE = {}


def _ensure_ntff_hook():
    """Install the antenv.axon_hooks shim if the image lacks it, wiring the
    NTFF profile hook straight to libaxon_pjrt.so (trace-only path)."""
    import sys
    import types
    try:
        from antenv.axon_hooks import get_axon_ntff_profile_hook  # noqa: F401
        return
    except ImportError:
        pass
    import antenv
    from trn_agent_boot.trn_boot import _ntff_profile_via_ctypes
    mod = types.ModuleType("antenv.axon_hooks")
    hook = [_ntff_profile_via_ctypes("/opt/axon/libaxon_pjrt.so")]
    mod.get_axon_ntff_profile_hook = lambda: hook[0]
    mod.set_axon_ntff_profile_hook = lambda h: hook.__setitem__(0, h)
    sys.modules["antenv.axon_hooks"] = mod
    antenv.axon_hooks = mod


def _build_program(G4, G2, G1, VCHUNKS):
    import concourse.bass as bass
    import concourse.tile as tile
    from concourse import bacc, mybir

    F16 = mybir.dt.float16
    F32 = mybir.dt.float32
    I32 = mybir.dt.int32
    MAX = mybir.AluOpType.max
    Gtot = G4 + G2 + G1
    NT = 4 * G4 + 2 * G2 + G1

    nc = bacc.Bacc(None)
    xs_d = nc.declare_dram_parameter("xs", [128, NT * 128], F16, isOutput=False)
    w8_d = nc.declare_dram_parameter("w8", [128, 512], F16, isOutput=False)
    if VCHUNKS:
        vg_d = nc.declare_dram_parameter("vgidx", [128, VCHUNKS], I32, isOutput=False)
        vs_d = nc.declare_dram_parameter("vsidx", [128, VCHUNKS], I32, isOutput=False)
    out_d = nc.declare_dram_parameter("out", [128, Gtot * 64], F32, isOutput=True)
    out_rows = out_d.ap().rearrange("p (g d) -> (p g) d", d=64)

    # (depth, n_groups, tile_base, group_base, act_drain_mod) per class;
    # act_drain_mod: for depth-1 groups, which take the ACT drain path.
    classes = [(4, G4, 0, 0), (2, G2, 4 * G4, G4), (1, G1, 4 * G4 + 2 * G2, G4 + G2)]

    with tile.TileContext(nc) as tc:
        with (
            tc.tile_pool(name="wp", bufs=1) as wp,
            tc.tile_pool(name="xsp", bufs=3) as xsp,
            tc.tile_pool(name="ps", bufs=8, space="PSUM") as ps,
            tc.tile_pool(name="sp", bufs=4) as sp,
            tc.tile_pool(name="tp", bufs=4) as tp,
            tc.tile_pool(name="stg", bufs=3) as stg,
            tc.tile_pool(name="vx", bufs=1) as vx,
        ):
            w8 = wp.tile([128, 512], F16)
            nc.sync.dma_start(out=w8[:], in_=w8_d[:])

            for depth, ngroups, tbase, gbase in classes:
                if ngroups == 0:
                    continue
                # units of 4 column-tiles = one [128, 2048] 4-bank PSUM tile
                gp_unit = 4 // depth            # groups per unit
                nunits = ngroups // gp_unit
                # cap chunk at 16 groups (PT/T1 sizing)
                units_per_chunk = max(min(CHUNK_TILES // 4, 16 // gp_unit), 1)
                for u0 in range(0, nunits, units_per_chunk):
                    u1 = min(u0 + units_per_chunk, nunits)
                    t0 = tbase + u0 * 4
                    ntile = (u1 - u0) * 4
                    ngc = (u1 - u0) * gp_unit   # groups in this chunk
                    xs = xsp.tile([128, CHUNK_TILES * 128], F16, tag="xs")
                    nc.sync.dma_start(
                        out=xs[:, : ntile * 128],
                        in_=xs_d[:, t0 * 128 : (t0 + ntile) * 128],
                    )
                    # per-group 512-wide (8 slots x 64 ch) pre-tree deposits
                    PT = stg.tile([128, 16 * 512], F16, tag="PT")
                    staging = stg.tile([128, 16 * 64], F16, tag="stg")
                    for ul in range(u1 - u0):
                        col = ul * 4 * 128
                        pslice = PT[:, ul * gp_unit * 512 :
                                    (ul + 1) * gp_unit * 512]
                        pp = ps.tile([128, 2048], F32, tag="psum4", bufs=2,
                                     name="pp")
                        for t in range(4):
                            nc.tensor.matmul(
                                pp[:, t * 512 : (t + 1) * 512],
                                xs[:, col + t * 128 : col + (t + 1) * 128],
                                w8[:], start=True, stop=True,
                            )
                        if depth == 4:
                            # tiles = [p0 p1 p2 p3] of one group
                            a02 = sp.tile([128, 1024], F16, tag="a02")
                            nc.scalar.activation(
                                out=a02[:].rearrange("p (a b) -> p a b", a=2),
                                in_=pp[:].rearrange("p (a c b) -> p a (c b)",
                                                    a=2, c=2)[:, :, 0:512],
                                func=mybir.ActivationFunctionType.Relu)
                            s1 = sp.tile([128, 1024], F16, tag="s1")
                            nc.vector.tensor_max(
                                s1[:].rearrange("p (a b) -> p a b", a=2),
                                pp[:].rearrange("p (a c b) -> p a (c b)",
                                                a=2, c=2)[:, :, 512:1024],
                                a02[:].rearrange("p (a b) -> p a b", a=2))
                            nc.vector.tensor_max(
                                pslice, s1[:, 0:512], s1[:, 512:1024])
                        elif depth == 2:
                            # tiles = [g0p0 g0p1 g1p0 g1p1]
                            if D2_ACT_BOTH_MOD and (u0 + ul) % D2_ACT_BOTH_MOD == 0:
                                aq = sp.tile([128, 2048], F16, tag="aq")
                                nc.scalar.activation(
                                    out=aq[:], in_=pp[:],
                                    func=mybir.ActivationFunctionType.Relu)
                                nc.vector.tensor_max(
                                    pslice.rearrange("p (a b) -> p a b", a=2),
                                    aq[:].rearrange("p (a c b) -> p a (c b)",
                                                    a=2, c=2)[:, :, 0:512],
                                    aq[:].rearrange("p (a c b) -> p a (c b)",
                                                    a=2, c=2)[:, :, 512:1024])
                            else:
                                a0p = sp.tile([128, 1024], F16, tag="a0p")
                                nc.scalar.activation(
                                    out=a0p[:].rearrange("p (a b) -> p a b", a=2),
                                    in_=pp[:].rearrange("p (a c b) -> p a (c b)",
                                                        a=2, c=2)[:, :, 0:512],
                                    func=mybir.ActivationFunctionType.Relu)
                                nc.vector.tensor_max(
                                    pslice.rearrange("p (a b) -> p a b", a=2),
                                    pp[:].rearrange("p (a c b) -> p a (c b)",
                                                    a=2, c=2)[:, :, 512:1024],
                                    a0p[:].rearrange("p (a b) -> p a b", a=2))
                        else:  # depth 1: four groups per unit, ACT -> PT direct
                            nc.scalar.activation(
                                out=pslice, in_=pp[:],
                                func=mybir.ActivationFunctionType.Relu)
                    # chunk-wide tree: PT [128, ngc*512] -> staging [128, ngc*64]
                    T1 = tp.tile([128, 16 * 256], F16, tag="T1")
                    nc.vector.tensor_max(
                        T1[:, : ngc * 256].rearrange("p (g b) -> p g b", b=256),
                        PT[:, : ngc * 512].rearrange("p (g c b) -> p g (c b)",
                                                     c=2, b=256)[:, :, 0:256],
                        PT[:, : ngc * 512].rearrange("p (g c b) -> p g (c b)",
                                                     c=2, b=256)[:, :, 256:512])
                    T2 = tp.tile([128, 16 * 128], F16, tag="T2")
                    nc.vector.tensor_max(
                        T2[:, : ngc * 128].rearrange("p (g b) -> p g b", b=128),
                        T1[:, : ngc * 256].rearrange("p (g c b) -> p g (c b)",
                                                     c=2, b=128)[:, :, 0:128],
                        T1[:, : ngc * 256].rearrange("p (g c b) -> p g (c b)",
                                                     c=2, b=128)[:, :, 128:256])
                    nc.vector.scalar_tensor_tensor(
                        out=staging[:, : ngc * 64].rearrange(
                            "p (g b) -> p g b", b=64),
                        in0=T2[:, : ngc * 128].rearrange(
                            "p (g c b) -> p g (c b)", c=2, b=64)[:, :, 0:64],
                        scalar=0.0,
                        in1=T2[:, : ngc * 128].rearrange(
                            "p (g c b) -> p g (c b)", c=2, b=64)[:, :, 64:128],
                        op0=MAX, op1=MAX)
                    # cast f16 -> f32 during the output DMA (SWDGE)
                    g0 = gbase + u0 * gp_unit
                    nc.gpsimd.dma_start(
                        out=out_d[:, g0 * 64 : (g0 + ngc) * 64],
                        in_=staging[:, : ngc * 64],
                    )

            if VCHUNKS:
                vg = vx.tile([128, VCHUNKS], I32)
                vs = vx.tile([128, VCHUNKS], I32)
                nc.sync.dma_start(out=vg[:], in_=vg_d[:])
                nc.sync.dma_start(out=vs[:], in_=vs_d[:])
                for b in range(VCHUNKS):
                    vrow = sp.tile([128, 64], F32, tag="vrow")
                    trow = sp.tile([128, 64], F32, tag="trow")
                    mrow = sp.tile([128, 64], F32, tag="mrow")
                    nc.gpsimd.indirect_dma_start(
                        out=vrow[:], out_offset=None,
                        in_=out_rows,
                        in_offset=bass.IndirectOffsetOnAxis(
                            ap=vg[:, b : b + 1], axis=0),
                    )
                    nc.gpsimd.indirect_dma_start(
                        out=trow[:], out_offset=None,
                        in_=out_rows,
                        in_offset=bass.IndirectOffsetOnAxis(
                            ap=vs[:, b : b + 1], axis=0),
                    )
                    nc.vector.tensor_max(mrow[:], vrow[:], trow[:])
                    nc.gpsimd.indirect_dma_start(
                        out=out_rows,
                        out_offset=bass.IndirectOffsetOnAxis(
                            ap=vs[:, b : b + 1], axis=0),
                        in_=mrow[:], in_offset=None,
                    )
    nc.finalize()
    return nc


def _depth_of(load):
    d = np.ones_like(load)
    d[load > 8] = 2
    d[load > 16] = 4
    return d


def kernel(group_features, pillar_set_indices, num_pillars, W, gamma, beta,
           running_mean, running_var):
    global LAST_RESULTS
    from concourse.bass_utils import run_bass_kernel_spmd

    x = np.ascontiguousarray(np.asarray(group_features, dtype=np.float32))
    idx = np.asarray(pillar_set_indices).astype(np.int64)
    M = int(num_pillars)
    P = x.shape[0]
    ppc = (M + N_CORES - 1) // N_CORES

    # ---- fold BN into the weights -----------------------------------------
    inv_std = np.asarray(gamma, np.float32) / np.sqrt(
        np.asarray(running_var, np.float32) + BN_EPS)
    Wt = np.zeros((F_PAD, C_OUT), np.float32)
    Wt[:C_IN] = np.asarray(W, np.float32) * inv_std[None, :]
    Wt[C_IN] = (np.asarray(beta, np.float32)
                - np.asarray(running_mean, np.float32) * inv_std)
    w8 = np.zeros((8, F_PAD, 512), np.float16)
    for r in range(8):
        w8[r, :, r * 64 : (r + 1) * 64] = Wt
    w8 = w8.reshape(128, 512)

    # ---- route points to pillar-owning cores ------------------------------
    order = np.argsort(idx, kind="stable")
    idx_s = idx[order]
    x_s = x[order]
    counts = np.bincount(idx_s, minlength=M)
    starts = np.zeros(M + 1, np.int64)
    np.cumsum(counts, out=starts[1:])
    rank = np.arange(P, dtype=np.int64) - starts[idx_s]

    # ---- per-core entity construction (class sizes first) -----------------
    percore = []
    N4 = N2 = N1 = NVB = 0
    for c in range(N_CORES):
        plo = c * ppc
        phi = min(plo + ppc, M)
        npil = phi - plo
        sl = slice(starts[plo], starts[phi])
        cnt = counts[plo:phi].astype(np.int64)
        # entities: chunk 0 of each pillar + overflow chunks (virtual)
        n_chunks = np.maximum((cnt + MAX_SLOTS - 1) // MAX_SLOTS, 1)
        nv = int((n_chunks - 1).sum())
        load_main = np.minimum(cnt, MAX_SLOTS)
        # virtual entity loads: chunks 1.. of overflowing pillars
        vp = np.nonzero(n_chunks > 1)[0]
        vload, vtgt, vlvl = [], [], []
        for p in vp:
            rem = cnt[p] - MAX_SLOTS
            lv = 0
            while rem > 0:
                vload.append(min(rem, MAX_SLOTS))
                vtgt.append(p)
                vlvl.append(lv)
                rem -= MAX_SLOTS
                lv += 1
        vload = np.array(vload, np.int64)
        load = np.concatenate([load_main, vload])
        depth = _depth_of(load)
        n4 = int((depth == 4).sum()); n2 = int((depth == 2).sum())
        n1 = int((depth == 1).sum())
        percore.append((plo, phi, sl, cnt, load, depth, vtgt, vlvl, nv))
        N4 = max(N4, n4); N2 = max(N2, n2); N1 = max(N1, n1)
        # fixup batches (each chain level padded to 128)
        if nv:
            lvl_arr = np.array(vlvl, np.int64)
            vb = sum((int((lvl_arr == lv).sum()) + 127) // 128
                     for lv in range(int(lvl_arr.max()) + 1))
            NVB = max(NVB, vb)
    G4 = (N4 + 127) // 128
    G2 = 2 * ((N2 + 255) // 256)            # even: depth-2 groups run in pairs
    G1 = 4 * ((N1 + 1 + 511) // 512)        # x4: depth-1 groups run in quads

    Gtot = G4 + G2 + G1
    NT = 4 * G4 + 2 * G2 + G1
    VCHUNKS = NVB

    # ---- per-core packing -------------------------------------------------
    in_maps = []
    unshard = []
    for c in range(N_CORES):
        plo, phi, sl, cnt, load, depth, vtgt, vlvl, nv = percore[c]
        npil = phi - plo
        ne = npil + nv
        # order entities: class 4, then 2, then 1 (stable)
        pos = np.zeros(ne, np.int64)
        i4 = np.nonzero(depth == 4)[0]
        i2 = np.nonzero(depth == 2)[0]
        i1 = np.nonzero(depth == 1)[0]
        pos[i4] = np.arange(len(i4))
        pos[i2] = G4 * 128 + np.arange(len(i2))
        pos[i1] = (G4 + G2) * 128 + np.arange(len(i1))
        # entity -> (tile base, j); groups are blocks of 128 positions
        g = pos // 128
        j = pos % 128
        dep_of_pos = np.where(g < G4, 4, np.where(g < G4 + G2, 2, 1))
        tbase = np.where(
            g < G4, g * 4,
            np.where(g < G4 + G2, 4 * G4 + (g - G4) * 2,
                     4 * G4 + 2 * G2 + (g - G4 - G2)))
        assert (dep_of_pos >= depth).all()

        # points -> (entity, slot)
        pid = idx_s[sl] - plo
        rk = rank[sl]
        chunk = rk // MAX_SLOTS
        kk = rk % MAX_SLOTS
        # virtual entity index for (pillar, chunk>=1)
        max_chain = (max(vlvl) + 1) if nv else 1
        virt_index = np.full((npil, max_chain), -1, np.int64)
        for v, (p, lv) in enumerate(zip(vtgt, vlvl)):
            virt_index[p, lv] = npil + v
        ent = np.where(chunk == 0, pid,
                       virt_index[pid, np.minimum(chunk - 1, max_chain - 1)])
        assert (ent >= 0).all()
        col = (tbase[ent] + kk // 8) * 128 + j[ent]
        row16 = kk % 8

        xs_dev = np.zeros((8, F_PAD, NT * 128), np.float16)
        xs_dev[row16, :C_IN, col] = x_s[sl].astype(np.float16)
        xs_dev[row16, C_IN, col] = 1.0
        xs_dev = xs_dev.reshape(128, NT * 128)

        im = {"xs": xs_dev, "w8": w8}
        if VCHUNKS:
            # device out row of entity q: j*Gtot + g; trash = first unused
            # depth-1 position (G1 reserves at least one spare)
            erow = j * Gtot + g
            trash = (len(i1) % 128) * Gtot + (G4 + G2 + len(i1) // 128)
            # order fixups by chain level, each level padded to 128
            gq_l, sq_l = [], []
            lvl_arr = np.array(vlvl, np.int64)
            for lv in range(int(lvl_arr.max()) + 1 if nv else 0):
                m = np.nonzero(lvl_arr == lv)[0]
                gl_ = erow[npil + m]
                sl_ = erow[np.array(vtgt, np.int64)[m]]
                pad = (-len(gl_)) % 128
                gq_l.append(np.pad(gl_, (0, pad), constant_values=trash))
                sq_l.append(np.pad(sl_, (0, pad), constant_values=trash))
            gq = (np.concatenate(gq_l) if gq_l else np.zeros(0, np.int64))
            sq = (np.concatenate(sq_l) if sq_l else np.zeros(0, np.int64))
            pad = VCHUNKS * 128 - len(gq)
            assert pad >= 0
            gq = np.pad(gq, (0, pad), constant_values=trash)
            sq = np.pad(sq, (0, pad), constant_values=trash)
            im["vgidx"] = np.ascontiguousarray(
                gq.reshape(VCHUNKS, 128).T.astype(np.int32))
            im["vsidx"] = np.ascontiguousarray(
                sq.reshape(VCHUNKS, 128).T.astype(np.int32))
        in_maps.append(im)
        unshard.append((plo, phi, g[:npil].copy(), j[:npil].copy()))

    # ---- build + run ------------------------------------------------------
    key = (G4, G2, G1, VCHUNKS)
    if key not in _PROGRAM_CACHE:
        _PROGRAM_CACHE[key] = _build_program(G4, G2, G1, VCHUNKS)
    nc = _PROGRAM_CACHE[key]

    trace = bool(int(os.environ.get("PILLAR_TRACE", "0")))
    if trace:
        _ensure_ntff_hook()
    res = run_bass_kernel_spmd(
        nc, in_maps, core_ids=list(range(N_CORES)), trace=trace,
        trace_cores=list(range(N_CORES)) if trace else None,
    )
    LAST_RESULTS = res

    # ---- unshard ----------------------------------------------------------
    out_full = np.zeros((M, C_OUT), np.float32)
    for c in range(N_CORES):
        plo, phi, gg, jj = unshard[c]
        dev = res.results[c]["out"].reshape(128, Gtot, C_OUT)
        out_full[plo:phi] = dev[jj, gg, :]
    return out_full


# revision 16
# speedup vs baseline: 1.1208x; 1.1208x over previous
"""PillarMaxPooling Trainium2 kernel (8 NeuronCores, SPMD).

Strategy
--------
Output (pillar) sharding: core c owns pillars [c*PPC, (c+1)*PPC).
Host-side prep is pure indexing/sharding work: points are routed to the
core that owns their pillar and packed into fixed per-pillar slot rows.
Pillars are stratified by point count into depth classes {1, 2, 4}
(8/16/32 slots); pillars with >32 points spill the excess into
"virtual pillar" entities combined on-device at the end.

BatchNorm folding: z = x @ (W * inv_std) + shift is one matmul via an
appended constant-1 feature carrying `shift`.  ReLU commutes with max,
and all-zero padding slots are exact neutral elements because
max(relu(a), 0) == relu(max(a, anything <= relu)).

Device program per core (identical program on all 8 cores):
  - xs  [128, NT*128] fp16 : slot features; entity (g,j) of depth d owns
        column-tiles [base(g) .. base(g)+d) at column j; each tile column
        packs 8 slots x 16 features down the 128-partition contraction.
  - w8  [128, 512] fp16 : block-diagonal folded weights; one matmul of an
        xs tile against w8 yields z for 8 slots x 64 channels (f32 PSUM).
  - per group: ACT relu-drains even PSUM tiles, DVE max-combines odd
        tiles, GPSIMD + DVE run the max tree -> [128, 64] f32 output rows.
"""

import os
import numpy as np

C_IN = 10
C_OUT = 64
N_CORES = 8
BN_EPS = 1e-3
F_PAD = 16            # features padded: 10 real + 1 const + 5 zero
MAX_SLOTS = 32        # slots per entity cap (depth class 4)
CHUNK_TILES = 32      # xs column-tiles per DMA chunk
D2_ACT_FIRST = 2      # first N d2 units of each chunk: ACT drains all four

LAST_RESULTS = None
_PROGRAM_CACHE = {}


def _ensure_ntff_hook():
    """Install the antenv.axon_hooks shim if the image lacks it, wiring the
    NTFF profile hook straight to libaxon_pjrt.so (trace-only path)."""
    import sys
    import types
    try:
        from antenv.axon_hooks import get_axon_ntff_profile_hook  # noqa: F401
        return
    except ImportError:
        pass
    import antenv
    from trn_agent_boot.trn_boot import _ntff_profile_via_ctypes
    mod = types.ModuleType("antenv.axon_hooks")
    hook = [_ntff_profile_via_ctypes("/opt/axon/libaxon_pjrt.so")]
    mod.get_axon_ntff_profile_hook = lambda: hook[0]
    mod.set_axon_ntff_profile_hook = lambda h: hook.__setitem__(0, h)
    sys.modules["antenv.axon_hooks"] = mod
    antenv.axon_hooks = mod


def _build_program(G4, G2, G1, VCHUNKS):
    import concourse.bass as bass
    import concourse.tile as tile
    from concourse import bacc, mybir

    F16 = mybir.dt.float16
    F32 = mybir.dt.float32
    I32 = mybir.dt.int32
    MAX = mybir.AluOpType.max
    Gtot = G4 + G2 + G1
    NT = 4 * G4 + 2 * G2 + G1

    nc = bacc.Bacc(None)
    xs_d = nc.declare_dram_parameter("xs", [128, NT * 128], F16, isOutput=False)
    w8_d = nc.declare_dram_parameter("w8", [128, 512], F16, isOutput=False)
    if VCHUNKS:
        vg_d = nc.declare_dram_parameter("vgidx", [128, VCHUNKS], I32, isOutput=False)
        vs_d = nc.declare_dram_parameter("vsidx", [128, VCHUNKS], I32, isOutput=False)
    out_d = nc.declare_dram_parameter("out", [128, Gtot * 64], F32, isOutput=True)
    out_rows = out_d.ap().rearrange("p (g d) -> (p g) d", d=64)

    # (depth, n_groups, tile_base, group_base, act_drain_mod) per class;
    # act_drain_mod: for depth-1 groups, which take the ACT drain path.
    classes = [(4, G4, 0, 0), (2, G2, 4 * G4, G4), (1, G1, 4 * G4 + 2 * G2, G4 + G2)]

    with tile.TileContext(nc) as tc:
        with (
            tc.tile_pool(name="wp", bufs=1) as wp,
            tc.tile_pool(name="xsp", bufs=3) as xsp,
            tc.tile_pool(name="ps", bufs=8, space="PSUM") as ps,
            tc.tile_pool(name="sp", bufs=4) as sp,
            tc.tile_pool(name="tp", bufs=4) as tp,
            tc.tile_pool(name="stg", bufs=3) as stg,
            tc.tile_pool(name="vx", bufs=1) as vx,
        ):
            w8 = wp.tile([128, 512], F16)
            nc.sync.dma_start(out=w8[:], in_=w8_d[:])

            for depth, ngroups, tbase, gbase in classes:
                if ngroups == 0:
                    continue
                # units of 4 column-tiles = one [128, 2048] 4-bank PSUM tile
                gp_unit = 4 // depth            # groups per unit
                nunits = ngroups // gp_unit
                # cap chunk at 16 groups (PT/T1 sizing)
                units_per_chunk = max(min(CHUNK_TILES // 4, 16 // gp_unit), 1)
                for u0 in range(0, nunits, units_per_chunk):
                    u1 = min(u0 + units_per_chunk, nunits)
                    t0 = tbase + u0 * 4
                    ntile = (u1 - u0) * 4
                    ngc = (u1 - u0) * gp_unit   # groups in this chunk
                    xs = xsp.tile([128, CHUNK_TILES * 128], F16, tag="xs")
                    nc.sync.dma_start(
                        out=xs[:, : ntile * 128],
                        in_=xs_d[:, t0 * 128 : (t0 + ntile) * 128],
                    )
                    # per-group 512-wide (8 slots x 64 ch) pre-tree deposits
                    PT = stg.tile([128, 16 * 512], F16, tag="PT")
                    staging = stg.tile([128, 16 * 64], F16, tag="stg")
                    for ul in range(u1 - u0):
                        col = ul * 4 * 128
                        pslice = PT[:, ul * gp_unit * 512 :
                                    (ul + 1) * gp_unit * 512]
                        pp = ps.tile([128, 2048], F32, tag="psum4", bufs=2,
                                     name="pp")
                        for t in range(4):
                            nc.tensor.matmul(
                                pp[:, t * 512 : (t + 1) * 512],
                                xs[:, col + t * 128 : col + (t + 1) * 128],
                                w8[:], start=True, stop=True,
                            )
                        if depth == 4:
                            # tiles = [p0 p1 p2 p3] of one group
                            a02 = sp.tile([128, 1024], F16, tag="a02")
                            nc.scalar.activation(
                                out=a02[:].rearrange("p (a b) -> p a b", a=2),
                                in_=pp[:].rearrange("p (a c b) -> p a (c b)",
                                                    a=2, c=2)[:, :, 0:512],
                                func=mybir.ActivationFunctionType.Relu)
                            s1 = sp.tile([128, 1024], F16, tag="s1")
                            nc.vector.tensor_max(
                                s1[:].rearrange("p (a b) -> p a b", a=2),
                                pp[:].rearrange("p (a c b) -> p a (c b)",
                                                a=2, c=2)[:, :, 512:1024],
                                a02[:].rearrange("p (a b) -> p a b", a=2))
                            nc.vector.tensor_max(
                                pslice, s1[:, 0:512], s1[:, 512:1024])
                        elif depth == 2:
                            # tiles = [g0p0 g0p1 g1p0 g1p1]
                            if ul < D2_ACT_FIRST:
                                aq = sp.tile([128, 2048], F16, tag="aq")
                                nc.scalar.activation(
                                    out=aq[:], in_=pp[:],
                                    func=mybir.ActivationFunctionType.Relu)
                                nc.vector.tensor_max(
                                    pslice.rearrange("p (a b) -> p a b", a=2),
                                    aq[:].rearrange("p (a c b) -> p a (c b)",
                                                    a=2, c=2)[:, :, 0:512],
                                    aq[:].rearrange("p (a c b) -> p a (c b)",
                                                    a=2, c=2)[:, :, 512:1024])
                            else:
                                a0p = sp.tile([128, 1024], F16, tag="a0p")
                                nc.scalar.activation(
                                    out=a0p[:].rearrange("p (a b) -> p a b", a=2),
                                    in_=pp[:].rearrange("p (a c b) -> p a (c b)",
                                                        a=2, c=2)[:, :, 0:512],
                                    func=mybir.ActivationFunctionType.Relu)
                                nc.vector.tensor_max(
                                    pslice.rearrange("p (a b) -> p a b", a=2),
                                    pp[:].rearrange("p (a c b) -> p a (c b)",
                                                    a=2, c=2)[:, :, 512:1024],
                                    a0p[:].rearrange("p (a b) -> p a b", a=2))
                        else:  # depth 1: four groups per unit, ACT -> PT direct
                            nc.scalar.activation(
                                out=pslice, in_=pp[:],
                                func=mybir.ActivationFunctionType.Relu)
                    # chunk-wide tree in two halves: PT -> staging
                    T1 = tp.tile([128, 16 * 256], F16, tag="T1")
                    T2 = tp.tile([128, 16 * 128], F16, tag="T2")
                    for h0, h1 in ((0, ngc // 2), (ngc // 2, ngc)):
                        nh = h1 - h0
                        if nh == 0:
                            continue
                        nc.vector.tensor_max(
                            T1[:, h0 * 256 : h1 * 256].rearrange(
                                "p (g b) -> p g b", b=256),
                            PT[:, h0 * 512 : h1 * 512].rearrange(
                                "p (g c b) -> p g (c b)", c=2, b=256)[:, :, 0:256],
                            PT[:, h0 * 512 : h1 * 512].rearrange(
                                "p (g c b) -> p g (c b)", c=2, b=256)[:, :, 256:512])
                        nc.vector.tensor_max(
                            T2[:, h0 * 128 : h1 * 128].rearrange(
                                "p (g b) -> p g b", b=128),
                            T1[:, h0 * 256 : h1 * 256].rearrange(
                                "p (g c b) -> p g (c b)", c=2, b=128)[:, :, 0:128],
                            T1[:, h0 * 256 : h1 * 256].rearrange(
                                "p (g c b) -> p g (c b)", c=2, b=128)[:, :, 128:256])
                        nc.vector.scalar_tensor_tensor(
                            out=staging[:, h0 * 64 : h1 * 64].rearrange(
                                "p (g b) -> p g b", b=64),
                            in0=T2[:, h0 * 128 : h1 * 128].rearrange(
                                "p (g c b) -> p g (c b)", c=2, b=64)[:, :, 0:64],
                            scalar=0.0,
                            in1=T2[:, h0 * 128 : h1 * 128].rearrange(
                                "p (g c b) -> p g (c b)", c=2, b=64)[:, :, 64:128],
                            op0=MAX, op1=MAX)
                    # cast f16 -> f32 during the output DMA (SWDGE)
                    g0 = gbase + u0 * gp_unit
                    nc.gpsimd.dma_start(
                        out=out_d[:, g0 * 64 : (g0 + ngc) * 64],
                        in_=staging[:, : ngc * 64],
                    )

            if VCHUNKS:
                vg = vx.tile([128, VCHUNKS], I32)
                vs = vx.tile([128, VCHUNKS], I32)
                nc.sync.dma_start(out=vg[:], in_=vg_d[:])
                nc.sync.dma_start(out=vs[:], in_=vs_d[:])
                for b in range(VCHUNKS):
                    vrow = sp.tile([128, 64], F32, tag="vrow")
                    trow = sp.tile([128, 64], F32, tag="trow")
                    mrow = sp.tile([128, 64], F32, tag="mrow")
                    nc.gpsimd.indirect_dma_start(
                        out=vrow[:], out_offset=None,
                        in_=out_rows,
                        in_offset=bass.IndirectOffsetOnAxis(
                            ap=vg[:, b : b + 1], axis=0),
                    )
                    nc.gpsimd.indirect_dma_start(
                        out=trow[:], out_offset=None,
                        in_=out_rows,
                        in_offset=bass.IndirectOffsetOnAxis(
                            ap=vs[:, b : b + 1], axis=0),
                    )
                    nc.vector.tensor_max(mrow[:], vrow[:], trow[:])
                    nc.gpsimd.indirect_dma_start(
                        out=out_rows,
                        out_offset=bass.IndirectOffsetOnAxis(
                            ap=vs[:, b : b + 1], axis=0),
                        in_=mrow[:], in_offset=None,
                    )
    nc.finalize()
    return nc


def _depth_of(load):
    d = np.ones_like(load)
    d[load > 8] = 2
    d[load > 16] = 4
    return d


def kernel(group_features, pillar_set_indices, num_pillars, W, gamma, beta,
           running_mean, running_var):
    global LAST_RESULTS
    from concourse.bass_utils import run_bass_kernel_spmd

    x = np.ascontiguousarray(np.asarray(group_features, dtype=np.float32))
    idx = np.asarray(pillar_set_indices).astype(np.int64)
    M = int(num_pillars)
    P = x.shape[0]
    ppc = (M + N_CORES - 1) // N_CORES

    # ---- fold BN into the weights -----------------------------------------
    inv_std = np.asarray(gamma, np.float32) / np.sqrt(
        np.asarray(running_var, np.float32) + BN_EPS)
    Wt = np.zeros((F_PAD, C_OUT), np.float32)
    Wt[:C_IN] = np.asarray(W, np.float32) * inv_std[None, :]
    Wt[C_IN] = (np.asarray(beta, np.float32)
                - np.asarray(running_mean, np.float32) * inv_std)
    w8 = np.zeros((8, F_PAD, 512), np.float16)
    for r in range(8):
        w8[r, :, r * 64 : (r + 1) * 64] = Wt
    w8 = w8.reshape(128, 512)

    # ---- route points to pillar-owning cores ------------------------------
    order = np.argsort(idx, kind="stable")
    idx_s = idx[order]
    x_s = x[order]
    counts = np.bincount(idx_s, minlength=M)
    starts = np.zeros(M + 1, np.int64)
    np.cumsum(counts, out=starts[1:])
    rank = np.arange(P, dtype=np.int64) - starts[idx_s]

    # ---- per-core entity construction (class sizes first) -----------------
    percore = []
    N4 = N2 = N1 = NVB = 0
    for c in range(N_CORES):
        plo = c * ppc
        phi = min(plo + ppc, M)
        npil = phi - plo
        sl = slice(starts[plo], starts[phi])
        cnt = counts[plo:phi].astype(np.int64)
        # entities: chunk 0 of each pillar + overflow chunks (virtual)
        n_chunks = np.maximum((cnt + MAX_SLOTS - 1) // MAX_SLOTS, 1)
        nv = int((n_chunks - 1).sum())
        load_main = np.minimum(cnt, MAX_SLOTS)
        # virtual entity loads: chunks 1.. of overflowing pillars
        vp = np.nonzero(n_chunks > 1)[0]
        vload, vtgt, vlvl = [], [], []
        for p in vp:
            rem = cnt[p] - MAX_SLOTS
            lv = 0
            while rem > 0:
                vload.append(min(rem, MAX_SLOTS))
                vtgt.append(p)
                vlvl.append(lv)
                rem -= MAX_SLOTS
                lv += 1
        vload = np.array(vload, np.int64)
        load = np.concatenate([load_main, vload])
        depth = _depth_of(load)
        n4 = int((depth == 4).sum()); n2 = int((depth == 2).sum())
        n1 = int((depth == 1).sum())
        percore.append((plo, phi, sl, cnt, load, depth, vtgt, vlvl, nv))
        N4 = max(N4, n4); N2 = max(N2, n2); N1 = max(N1, n1)
        # fixup batches (each chain level padded to 128)
        if nv:
            lvl_arr = np.array(vlvl, np.int64)
            vb = sum((int((lvl_arr == lv).sum()) + 127) // 128
                     for lv in range(int(lvl_arr.max()) + 1))
            NVB = max(NVB, vb)
    G4 = (N4 + 127) // 128
    G2 = 2 * ((N2 + 255) // 256)            # even: depth-2 groups run in pairs
    G1 = 4 * ((N1 + 1 + 511) // 512)        # x4: depth-1 groups run in quads

    Gtot = G4 + G2 + G1
    NT = 4 * G4 + 2 * G2 + G1
    VCHUNKS = NVB

    # ---- per-core packing -------------------------------------------------
    in_maps = []
    unshard = []
    for c in range(N_CORES):
        plo, phi, sl, cnt, load, depth, vtgt, vlvl, nv = percore[c]
        npil = phi - plo
        ne = npil + nv
        # order entities: class 4, then 2, then 1 (stable)
        pos = np.zeros(ne, np.int64)
        i4 = np.nonzero(depth == 4)[0]
        i2 = np.nonzero(depth == 2)[0]
        i1 = np.nonzero(depth == 1)[0]
        pos[i4] = np.arange(len(i4))
        pos[i2] = G4 * 128 + np.arange(len(i2))
        pos[i1] = (G4 + G2) * 128 + np.arange(len(i1))
        # entity -> (tile base, j); groups are blocks of 128 positions
        g = pos // 128
        j = pos % 128
        dep_of_pos = np.where(g < G4, 4, np.where(g < G4 + G2, 2, 1))
        tbase = np.where(
            g < G4, g * 4,
            np.where(g < G4 + G2, 4 * G4 + (g - G4) * 2,
                     4 * G4 + 2 * G2 + (g - G4 - G2)))
        assert (dep_of_pos >= depth).all()

        # points -> (entity, slot)
        pid = idx_s[sl] - plo
        rk = rank[sl]
        chunk = rk // MAX_SLOTS
        kk = rk % MAX_SLOTS
        # virtual entity index for (pillar, chunk>=1)
        max_chain = (max(vlvl) + 1) if nv else 1
        virt_index = np.full((npil, max_chain), -1, np.int64)
        for v, (p, lv) in enumerate(zip(vtgt, vlvl)):
            virt_index[p, lv] = npil + v
        ent = np.where(chunk == 0, pid,
                       virt_index[pid, np.minimum(chunk - 1, max_chain - 1)])
        assert (ent >= 0).all()
        col = (tbase[ent] + kk // 8) * 128 + j[ent]
        row16 = kk % 8

        xs_dev = np.zeros((8, F_PAD, NT * 128), np.float16)
        xs_dev[row16, :C_IN, col] = x_s[sl].astype(np.float16)
        xs_dev[row16, C_IN, col] = 1.0
        xs_dev = xs_dev.reshape(128, NT * 128)

        im = {"xs": xs_dev, "w8": w8}
        if VCHUNKS:
            # device out row of entity q: j*Gtot + g; trash = first unused
            # depth-1 position (G1 reserves at least one spare)
            erow = j * Gtot + g
            trash = (len(i1) % 128) * Gtot + (G4 + G2 + len(i1) // 128)
            # order fixups by chain level, each level padded to 128
            gq_l, sq_l = [], []
            lvl_arr = np.array(vlvl, np.int64)
            for lv in range(int(lvl_arr.max()) + 1 if nv else 0):
                m = np.nonzero(lvl_arr == lv)[0]
                gl_ = erow[npil + m]
                sl_ = erow[np.array(vtgt, np.int64)[m]]
                pad = (-len(gl_)) % 128
                gq_l.append(np.pad(gl_, (0, pad), constant_values=trash))
                sq_l.append(np.pad(sl_, (0, pad), constant_values=trash))
            gq = (np.concatenate(gq_l) if gq_l else np.zeros(0, np.int64))
            sq = (np.concatenate(sq_l) if sq_l else np.zeros(0, np.int64))
            pad = VCHUNKS * 128 - len(gq)
            assert pad >= 0
            gq = np.pad(gq, (0, pad), constant_values=trash)
            sq = np.pad(sq, (0, pad), constant_values=trash)
            im["vgidx"] = np.ascontiguousarray(
                gq.reshape(VCHUNKS, 128).T.astype(np.int32))
            im["vsidx"] = np.ascontiguousarray(
                sq.reshape(VCHUNKS, 128).T.astype(np.int32))
        in_maps.append(im)
        unshard.append((plo, phi, g[:npil].copy(), j[:npil].copy()))

    # ---- build + run ------------------------------------------------------
    key = (G4, G2, G1, VCHUNKS)
    if key not in _PROGRAM_CACHE:
        _PROGRAM_CACHE[key] = _build_program(G4, G2, G1, VCHUNKS)
    nc = _PROGRAM_CACHE[key]

    trace = bool(int(os.environ.get("PILLAR_TRACE", "0")))
    if trace:
        _ensure_ntff_hook()
    res = run_bass_kernel_spmd(
        nc, in_maps, core_ids=list(range(N_CORES)), trace=trace,
        trace_cores=list(range(N_CORES)) if trace else None,
    )
    LAST_RESULTS = res

    # ---- unshard ----------------------------------------------------------
    out_full = np.zeros((M, C_OUT), np.float32)
    for c in range(N_CORES):
        plo, phi, gg, jj = unshard[c]
        dev = res.results[c]["out"].reshape(128, Gtot, C_OUT)
        out_full[plo:phi] = dev[jj, gg, :]
    return out_full
